# revision 48
# baseline (speedup 1.0000x reference)
"""Involution (B=4, C=256, H=W=56, K=7, G=16, reduction=4) on 8 trn2 NeuronCores.

Sharding: 8 shards = (batch b in 0..3) x (h-half in 0..1); each core computes
its [256, 28, 56] output slab from a [256, 34, 62] zero-padded input slab.

Per-core pipeline (engine-balanced):
  1. stage1 (PE + ACT): t_ext = [relu(bn(W1 @ x)); ones] in bf16, BN folded
     into W1/b1 on host.
  2. per-pixel kernels, two providers balanced across engines:
     - D-path (tap rows 0-1): compact matmul (PE) -> ACT cast -> DRAM
       scratch -> per-tap replicated-read DMA broadcast to [128, 7, 1568]
       bf16 tiles (DMA engines; ~115 GB/s effective, so only 2 rows).
     - A-path (tap rows 2-6): per-tap PE broadcast matmul (channel-replicated
       lhsT) -> ACT PSUM->SBUF bf16 cast.
  3. involution (DVE + PE): prod = x_win * wbc on DVE (bf16 2x mode);
     PE accumulates all products into a fp32 PSUM accumulator via identity
     matmuls (start at tap 0); ACT copies the result to SBUF for output DMA.
No GPSIMD (it contends with DVE's SBUF port); all DMA issue on the sync
queue (scalar-queue DMA issue corrupts concurrent ACT copies).
"""

import numpy as np
import ml_dtypes
from contextlib import ExitStack

import concourse.bass as bass
import concourse.bacc as bacc
import concourse.tile as tile
from concourse import mybir
from concourse.bass_utils import run_bass_kernel_spmd

BF16 = ml_dtypes.bfloat16

B, C, H, W = 4, 256, 56, 56
KK, G, PAD = 7, 16, 3
Cr, Cg = 64, 16
EPS = 1e-5
HH = H // 2              # 28 rows per h-half shard
PH, PW = HH + 2 * PAD, W + 2 * PAD   # 34, 62 padded slab dims
PWo = PW - 2             # odd-shifted slab width (60)
NPIX = HH * W            # 1568 output pixels per shard
NCORES = 8
NM = 7                   # tap blocks of 7 taps each (block m = tap row i=m)
DMA_MS = (0, 1)          # tap rows broadcast via DMA replicated reads
ACT_MS = tuple(m for m in range(NM) if m not in DMA_MS)
ND = len(DMA_MS)
NACT = len(ACT_MS) * KK * 2          # ACT-path (tap, half) pairs
MM_CHUNKS = [(0, 512), (512, 512), (1024, 512), (1536, 32)]
HPIX = NPIX // 2         # 784: A-path pw tile free size

_CACHE = {}

# set by test.py to collect a hardware profile
TRACE = False
LAST_RESULT = None


def _build_nc():
    nc = bacc.Bacc("TRN2", target_bir_lowering=False, debug=False,
                   num_devices=NCORES)

    f32 = mybir.dt.float32
    bf16 = mybir.dt.bfloat16

    x_d = nc.declare_dram_parameter("x", [2, 128, PH, PW], bf16, isOutput=False)
    xo_d = nc.declare_dram_parameter("xo", [2, 128, PH, PWo], bf16,
                                     isOutput=False)
    w1t_d = nc.declare_dram_parameter("w1t", [2, 128, Cr], bf16, isOutput=False)
    b1p_d = nc.declare_dram_parameter("b1p", [Cr, 1], f32, isOutput=False)
    w2c_d = nc.declare_dram_parameter("w2c", [Cr + 1, ND, 112], bf16,
                                      isOutput=False)
    w2bc_d = nc.declare_dram_parameter("w2bc", [Cr + 1, NACT, 128],
                                       bf16, isOutput=False)
    id_d = nc.declare_dram_parameter("ident", [128, 128], bf16, isOutput=False)
    out_d = nc.declare_dram_parameter("out", [C, HH, W], f32, isOutput=True)

    with tile.TileContext(nc) as tc, ExitStack() as ctx:
        const = ctx.enter_context(tc.tile_pool(name="const", bufs=1))
        xpool = ctx.enter_context(tc.tile_pool(name="x", bufs=1))
        tpool = ctx.enter_context(tc.tile_pool(name="t", bufs=1))
        dram = ctx.enter_context(tc.tile_pool(name="wdram", bufs=1,
                                              space="DRAM"))

        # stage-1-critical loads first (x, w1t, b1p): the big w2bc constant
        # would otherwise block them on the FIFO sync queue for ~8us.
        w1t_sb = const.tile([128, 2, Cr], bf16)
        for ch in range(2):
            nc.sync.dma_start(w1t_sb[:, ch, :], w1t_d[ch])
        b1p_sb = const.tile([Cr, 1], f32)
        nc.sync.dma_start(b1p_sb[:], b1p_d[:])
        w2c_sb = const.tile([Cr + 1, ND, 112], bf16)
        nc.sync.dma_start(w2c_sb[:], w2c_d[:])

        # input slabs (already zero-padded + bf16-cast on host); xo is the
        # same shifted left one column so odd-j tap windows stay 4B-aligned.
        x_sb, xo_sb = [], []
        for ch in range(2):
            xb = xpool.tile([128, PH, PW], bf16, tag=f"xb{ch}")
            nc.sync.dma_start(xb[:], x_d[ch])
            x_sb.append(xb)
            xo = xpool.tile([128, PH, PWo], bf16, tag=f"xo{ch}")
            nc.sync.dma_start(xo[:], xo_d[ch])
            xo_sb.append(xo)

        w2bc_sb = const.tile([Cr + 1, NACT, 128], bf16)
        nc.sync.dma_start(w2bc_sb[:], w2bc_d[:])
        id_sb = const.tile([128, 128], bf16)
        nc.sync.dma_start(id_sb[:], id_d[:])

        # ---- stage 1: t_ext = [relu(W1p @ x + b1p); ones] in bf16 ----
        t_ext = tpool.tile([Cr + 1, NPIX], bf16)
        nc.vector.memset(t_ext[Cr:Cr + 1, :], 1.0)
        with tc.tile_pool(name="psum_t", bufs=2,
                          space=bass.MemorySpace.PSUM) as psum_t:
            NRC = 7          # 7 rows x 56 cols = 392 <= 512 (one bank)
            for q in range(HH // NRC):
                pt = psum_t.tile([Cr, NRC * W], f32)
                for ch in range(2):
                    rhs = x_sb[ch][:, PAD + q * NRC:PAD + (q + 1) * NRC,
                                   PAD:PAD + W]
                    nc.tensor.matmul(pt[:], w1t_sb[:, ch, :], rhs,
                                     start=(ch == 0), stop=(ch == 1))
                nc.scalar.activation(
                    t_ext[0:Cr, q * NRC * W:(q + 1) * NRC * W],
                    pt[:], mybir.ActivationFunctionType.Relu,
                    bias=b1p_sb[:], scale=1.0)

        # ---- stage 2 (D-path rows only): compact kernels -> DRAM scratch ----
        # w_dram[mi]: [16 groups, 7 taps, 1568]; compact matmul emits
        # partition p = 7*g + kk, i.e. exactly that layout.
        w_dram = [dram.tile([G, KK, NPIX], bf16, tag=f"wd{mi}", bufs=1,
                            name=f"wd{mi}")
                  for mi in range(ND)]
        with tc.tile_pool(name="psum_c", bufs=2,
                          space=bass.MemorySpace.PSUM) as psum_c, \
                tc.tile_pool(name="wq", bufs=2) as wqp:
            for mi in range(ND):
                pc = psum_c.tile([112, NPIX], f32)
                for (o, n) in MM_CHUNKS:
                    nc.tensor.matmul(pc[:, o:o + n], w2c_sb[:, mi, :],
                                     t_ext[:, o:o + n], start=True, stop=True)
                wq = wqp.tile([112, NPIX], bf16, tag="wq")
                nc.scalar.copy(wq[:], pc[:])
                nc.sync.dma_start(
                    w_dram[mi][:].rearrange("g k n -> (g k) n"), wq[:])

        # ---- stage 3: involution ----
        accp = ctx.enter_context(
            tc.tile_pool(name="acc", bufs=1, space=bass.MemorySpace.PSUM))
        pwp = ctx.enter_context(
            tc.tile_pool(name="pw", bufs=2, space=bass.MemorySpace.PSUM))
        wbcp = ctx.enter_context(tc.tile_pool(name="wbc", bufs=4))
        wbsp = ctx.enter_context(tc.tile_pool(name="wbs", bufs=9))
        prodp = ctx.enter_context(tc.tile_pool(name="prod", bufs=6))
        outp = ctx.enter_context(tc.tile_pool(name="outp", bufs=2))

        # Prefetch every DMA broadcast up front: the replicated reads are
        # slow (~115 GB/s) but run in the background under the A/V stretches.
        wbc_map = {}
        for ch in range(2):
            for mi, m in enumerate(DMA_MS):
                wbc = wbcp.tile([128, KK, NPIX], bf16, tag="wbc", name="wbc")
                for gl in range(8):
                    src = w_dram[mi][8 * ch + gl].unsqueeze(0) \
                        .to_broadcast([16, KK, NPIX])
                    nc.sync.dma_start(wbc[16 * gl:16 * (gl + 1), :, :], src)
                wbc_map[(ch, m)] = wbc

        # Tap schedule: interleave D-taps (DVE-only) and V-taps (DVE reads
        # PSUM directly, no ACT) between A-taps so ACT, PE, and DVE all stay
        # fed; movers are emitted LOOKAHEAD taps ahead of consumption to
        # dodge head-of-line blocking on the in-order PE queue.
        V_SET = {(m, 1) for m in ACT_MS} | {(3, 5), (5, 5)}
        a_s = [(m, kk) for m in ACT_MS for kk in range(KK)
               if (m, kk) not in V_SET]
        d_s = [(m, kk) for m in DMA_MS for kk in range(KK)]
        v_s = sorted(V_SET)
        # first 14 slots avoid D-taps (their DMA tiles land ~60us in);
        # then 2-in-5 D density keeps ACT fed while draining the DMA tiles.
        K_LIST = []
        for r in range(2):
            for src_list in (a_s, a_s, a_s, v_s, a_s, a_s, a_s):
                if src_list:
                    K_LIST.append(src_list.pop(0))
        for r in range(KK):
            for src_list in (d_s, a_s, d_s, a_s, v_s):
                if src_list:
                    K_LIST.append(src_list.pop(0))
        K_LIST += a_s + d_s + v_s
        assert len(K_LIST) == KK * NM
        LOOKAHEAD = 7

        for ch in range(2):
            acc = accp.tile([128, NPIX], f32, tag="acc")
            wv_map = {}

            def emit_mover(q, ch=ch, wv_map=wv_map):
                m, kk = K_LIST[q]
                if m in DMA_MS or (m, kk) in V_SET:
                    return
                idx = (ACT_MS.index(m) * KK + kk) * 2 + ch
                wv = wbsp.tile([128, NPIX], bf16, tag="wbs", name="wv")
                for h in range(2):
                    pw = pwp.tile([128, HPIX], f32, tag="pw", name="pw")
                    o0 = h * HPIX
                    for (o, n) in ((o0, 512), (o0 + 512, HPIX - 512)):
                        nc.tensor.matmul(pw[:, o - o0:o - o0 + n],
                                         w2bc_sb[:, idx, :],
                                         t_ext[:, o:o + n],
                                         start=True, stop=True)
                    nc.scalar.copy(wv[:, o0:o0 + HPIX], pw[:])
                wv_map[q] = wv

            emitted = set()
            pending = None
            n_acc = 0
            for q in range(len(K_LIST)):
                for qq in range(q, min(q + 1 + LOOKAHEAD, len(K_LIST))):
                    if qq not in emitted:
                        emit_mover(qq)
                        emitted.add(qq)
                m, kk = K_LIST[q]
                i, j = m, kk
                if j % 2 == 0:
                    xwin = x_sb[ch][:, i:i + HH, j:j + W]
                else:
                    xwin = xo_sb[ch][:, i:i + HH, j - 1:j - 1 + W]
                prod = prodp.tile([128, NPIX], bf16, tag="prod")
                pview = prod[:].rearrange("p (h w) -> p h w", h=HH)
                if (m, kk) in V_SET:
                    # V-path: PE broadcast, DVE multiplies from PSUM (1x)
                    idx = (ACT_MS.index(m) * KK + kk) * 2 + ch
                    for h in range(2):
                        pw = pwp.tile([128, HPIX], f32, tag="pw", name="pw")
                        o0 = h * HPIX
                        for (o, n) in ((o0, 512), (o0 + 512, HPIX - 512)):
                            nc.tensor.matmul(pw[:, o - o0:o - o0 + n],
                                             w2bc_sb[:, idx, :],
                                             t_ext[:, o:o + n],
                                             start=True, stop=True)
                        nc.vector.tensor_mul(
                            pview[:, 14 * h:14 * (h + 1), :],
                            xwin[:, 14 * h:14 * (h + 1), :],
                            pw[:].rearrange("p (h w) -> p h w", h=HH // 2))
                else:
                    if m in DMA_MS:
                        wv = wbc_map[(ch, m)][:, kk, :]
                    else:
                        wv = wv_map.pop(q)
                    nc.vector.tensor_mul(
                        pview, xwin, wv.rearrange("p (h w) -> p h w", h=HH))
                for (o, n) in MM_CHUNKS:
                    nc.tensor.matmul(acc[:, o:o + n], id_sb[:],
                                     prod[:, o:o + n],
                                     start=(n_acc == 0),
                                     stop=(q == len(K_LIST) - 1))
                n_acc += 1
            of = outp.tile([128, NPIX], f32, tag="of")
            oview = of[:].rearrange("p (h w) -> p h w", h=HH)
            for h in range(2):
                nc.scalar.copy(of[:, h * HPIX:(h + 1) * HPIX],
                               acc[:, h * HPIX:(h + 1) * HPIX])
                nc.sync.dma_start(
                    out_d[ch * 128:(ch + 1) * 128, 14 * h:14 * (h + 1), :],
                    oview[:, 14 * h:14 * (h + 1), :])

    nc.compile()
    return nc


def _prep_host_inputs(inputs, W1, b1, gamma, beta, mean, var, W2, b2):
    """Fold BN into W1/b1; build compact + broadcast W2 arrangements."""
    scale = gamma / np.sqrt(var + EPS)
    shift = beta - mean * scale
    W1p = W1 * scale[:, None]
    b1p = (b1 * scale + shift).astype(np.float32).reshape(Cr, 1)
    w1t = np.ascontiguousarray(W1p.T.reshape(2, 128, Cr)).astype(BF16)

    # W2e[r, g*49+k] with bias row
    W2e = np.zeros((Cr + 1, G * KK * KK), np.float32)
    W2e[0:Cr] = W2.T
    W2e[Cr] = b2

    # compact lhsT per D-path block: column p = 7*g + kk -> (g, tap 7m+kk)
    w2c = np.zeros((Cr + 1, ND, 112), np.float32)
    for mi, m in enumerate(DMA_MS):
        for g in range(G):
            for kk in range(KK):
                w2c[:, mi, KK * g + kk] = W2e[:, g * KK * KK + KK * m + kk]
    w2c = w2c.astype(BF16)

    # A-path broadcast lhsT: idx = (mi*7+kk)*2+ch,
    # column p -> group 8*ch + p//16, tap 7*m+kk
    w2bc = np.zeros((Cr + 1, NACT, 128), np.float32)
    for mi, m in enumerate(ACT_MS):
        for kk in range(KK):
            for ch in range(2):
                idx = (mi * KK + kk) * 2 + ch
                gcol = (8 * ch + np.arange(128) // 16) * KK * KK + KK * m + kk
                w2bc[:, idx, :] = W2e[:, gcol]
    w2bc = w2bc.astype(BF16)

    ident = np.eye(128, dtype=np.float32).astype(BF16)

    # per-core padded input slabs, bf16 + odd-shifted copy
    xs, xos = [], []
    for core in range(NCORES):
        b, hf = core // 2, core % 2
        slab = np.zeros((C, PH, PW), np.float32)
        r0 = hf * HH - PAD
        r1 = r0 + PH
        v0, v1 = max(r0, 0), min(r1, H)
        slab[:, v0 - r0:v1 - r0, PAD:PAD + W] = inputs[b, :, v0:v1, :]
        slab = slab.astype(BF16)
        xs.append(np.ascontiguousarray(slab.reshape(2, 128, PH, PW)))
        xos.append(np.ascontiguousarray(
            slab[:, :, 1:PW - 1].reshape(2, 128, PH, PWo)))
    return xs, xos, w1t, b1p, w2c, w2bc, ident


def kernel(inputs, W1, b1, gamma, beta, mean, var, W2, b2):
    global LAST_RESULT
    inputs = np.asarray(inputs, np.float32)
    if "nc" not in _CACHE:
        _CACHE["nc"] = _build_nc()
    nc = _CACHE["nc"]

    xs, xos, w1t, b1p, w2c, w2bc, ident = _prep_host_inputs(
        inputs, np.asarray(W1, np.float32), np.asarray(b1, np.float32),
        np.asarray(gamma, np.float32), np.asarray(beta, np.float32),
        np.asarray(mean, np.float32), np.asarray(var, np.float32),
        np.asarray(W2, np.float32), np.asarray(b2, np.float32))

    in_maps = [{"x": xs[core], "xo": xos[core], "w1t": w1t, "b1p": b1p,
                "w2c": w2c, "w2bc": w2bc, "ident": ident}
               for core in range(NCORES)]
    res = run_bass_kernel_spmd(nc, in_maps, list(range(NCORES)), trace=TRACE)
    LAST_RESULT = res

    out = np.empty((B, C, H, W), np.float32)
    for core in range(NCORES):
        b, hf = core // 2, core % 2
        out[b, :, hf * HH:(hf + 1) * HH, :] = res.results[core]["out"]
    return out
